# revision 75
# baseline (speedup 1.0000x reference)
"""Multi-head causal attention (B=4, S=2048, D=1024, H=16, HD=64) on 8 TRN2 cores.

Sharding: core c handles (batch b = c//2, head-group hg = c%2 of 8 heads).
Each core computes QKV projections for its 512-dim head slice, transposed-layout
causal attention, and a partial output projection. Host sums the two head-group
partials per batch and adds the bias.

Per-core pipeline (v2 — software-pipelined across pieces):
  - x fed pre-transposed: xT [1024, 2048]; QT/KT/V via bf16 K=128 matmuls.
  - KT per pair stored bf16 [128, S]: head-even dims on partitions 0-63,
    head-odd on 64-127 (no zero padding); scores run as K=64 matmuls with
    base-partition-64 operands for the odd head (tile_position inferred).
  - Exact causal ragged windows SKS=[0,128,256,384]; a single upper-triangular
    maskA multiplies the 128-wide diagonal block of each kept chunk.
  - P^T = exp(ST/8) on ScalarE; ctx^T = V_aug^T @ P^T with V_aug = [V_h | 1]
    (M=65): row 64 accumulates softmax denominators r[q] for free.
  - normalize: r row copied to SBUF (f32r), broadcast to partitions 0-63 by a
    K=1 f32r matmul (213ns) into a st-ring PSUM tile, reciprocal + muls on
    DVE; head-odd ctx rows moved to partitions 64-127 by one SBUF-SBUF DMA
    issued via GpSimd's software DGE.
  - out[s, :] partial = sum_pairs ctxT_pair.T @ Wo_pair (K=128, moving f32r).
  - Engine schedule: QKV matmuls of piece p+1 and the output projection of
    piece p-1 are woven between attention matmuls of piece p in program order
    so TensorE never starves while ScalarE runs the exp chain.
"""

import os
import numpy as np

import concourse.bass as bass
import concourse.mybir as mybir
from concourse import bacc
import concourse.tile as tile
from concourse.bass_utils import run_bass_kernel_spmd

F32 = mybir.dt.float32
F32R = mybir.dt.float32r
BF16 = mybir.dt.bfloat16
EXP = mybir.ActivationFunctionType.Exp

P = 128
S = 2048
DIN = 1024
DH = 512          # per-core d_out slice (8 heads x 64)
NKC = DIN // P    # 8 contraction chunks
NPAIR = 4         # head pairs per core
NPIECE = S // 512 # 4 q pieces
W = 512

# exact causal ragged start offsets for diagonal chunks (krel = chunk - 4*piece)
SKS = [0, 128, 256, 384]

MM_NS = 0.4167  # PE ns/col, used only for weave pacing


def build_program() -> bass.Bass:
    nc = bacc.Bacc("TRN2", target_bir_lowering=False)

    xT_d = nc.dram_tensor("xT", [DIN, S], BF16, kind="ExternalInput")
    wq_d = nc.dram_tensor("wq", [DIN, DH], BF16, kind="ExternalInput")
    wk_d = nc.dram_tensor("wk", [DIN, DH], BF16, kind="ExternalInput")
    wv_d = nc.dram_tensor("wv", [DIN, DH], BF16, kind="ExternalInput")
    wo_d = nc.dram_tensor("wo", [DH, DIN], F32, kind="ExternalInput")
    maskA_d = nc.dram_tensor("maskA", [P, P], BF16, kind="ExternalInput")
    ones_d = nc.dram_tensor("ones", [P, 64], F32, kind="ExternalInput")
    out_d = nc.dram_tensor("out", [S, DIN], F32, kind="ExternalOutput")

    with tile.TileContext(nc) as tc:
        with (
            tc.tile_pool(name="consts", bufs=1) as consts,
            tc.tile_pool(name="xtp", bufs=4) as xtp,
            tc.tile_pool(name="qtp", bufs=3) as qtp,
            tc.tile_pool(name="ptp", bufs=6) as ptp,
            tc.tile_pool(name="ctxtp", bufs=2) as ctxtp,
            tc.tile_pool(name="rp", bufs=3) as rp,
            tc.tile_pool(name="osbp", bufs=6) as osbp,
            tc.tile_pool(name="ps_st", bufs=2, space="PSUM") as ps_st,
            tc.tile_pool(name="ps_ctx", bufs=1, space="PSUM") as ps_ctx,
            tc.tile_pool(name="ps_mm", bufs=2, space="PSUM") as ps_mm,
        ):
            # ---- input/weight DMAs, batched one per tensor, issued on SP ----
            xT_r = xT_d.rearrange("(kc p) s -> p kc s", p=P)
            xts = []
            wq_sb = consts.tile([P, NKC, DH], BF16)
            wk_sb = consts.tile([P, NKC, DH], BF16)
            wv_sb = consts.tile([P, NKC, DH], BF16)
            wo_sb = consts.tile([P, NPAIR, DIN], F32R)
            maskA = consts.tile([P, P], BF16)

            wq_r = wq_d.rearrange("(kc p) d -> p kc d", p=P)
            nc.sync.dma_start(
                wq_sb[:, 0 : NKC // 2, 0:P], wq_r[:, 0 : NKC // 2, 0:P]
            )
            # piece-0 x split so the first Q matmuls start as soon as the
            # first contraction chunks land
            xt0a1 = xtp.tile([P, 2, W], BF16, tag="xa1", name="xt0a1")
            xt0a2 = xtp.tile([P, 2, W], BF16, tag="xa2", name="xt0a2")
            xt0b = xtp.tile([P, NKC // 2, W], BF16, tag="xtb", name="xt0b")
            nc.sync.dma_start(xt0a1[:], xT_r[:, 0:2, 0:W])
            nc.sync.dma_start(xt0a2[:], xT_r[:, 2:4, 0:W])
            nc.sync.dma_start(
                wq_sb[:, NKC // 2 : NKC, 0:P], wq_r[:, NKC // 2 : NKC, 0:P]
            )
            nc.sync.dma_start(xt0b[:], xT_r[:, NKC // 2 : NKC, 0:W])
            xts.append((xt0a1, xt0a2, xt0b))
            for j in range(1, NPAIR):
                nc.sync.dma_start(
                    wq_sb[:, :, P * j : P * j + P], wq_r[:, :, P * j : P * j + P]
                )
            nc.sync.dma_start(wv_sb[:], wv_d.rearrange("(kc p) d -> p kc d", p=P))
            nc.sync.dma_start(wk_sb[:], wk_d.rearrange("(kc p) d -> p kc d", p=P))
            for p in range(1, NPIECE):
                xa1 = xtp.tile([P, 2, W], BF16, tag="xa1", name=f"xt{p}a1")
                xa2 = xtp.tile([P, 2, W], BF16, tag="xa2", name=f"xt{p}a2")
                xb = xtp.tile([P, NKC // 2, W], BF16, tag="xtb", name=f"xt{p}b")
                nc.sync.dma_start(xa1[:], xT_r[:, 0:2, W * p : W * p + W])
                nc.sync.dma_start(xa2[:], xT_r[:, 2:4, W * p : W * p + W])
                nc.sync.dma_start(xb[:], xT_r[:, NKC // 2 : NKC, W * p : W * p + W])
                xts.append((xa1, xa2, xb))
            nc.sync.dma_start(
                wo_sb[:], wo_d.rearrange("(g p) d -> p g d", p=P).bitcast(F32R)
            )
            nc.sync.dma_start(maskA[:], maskA_d[:])
            ones_r = consts.tile([P, 64], F32R)
            nc.sync.dma_start(ones_r[:], ones_d[:].bitcast(F32R))

            def xt_ap(q, kc):
                if kc < 2:
                    return xts[q][0][:, kc, :]
                if kc < 4:
                    return xts[q][1][:, kc - 2, :]
                return xts[q][2][:, kc - 4, :]

            # KT per pair [128, S]: head-even dims on partitions 0-63,
            # head-odd on 64-127.  V per piece [128, s-chunk(4), head(8), 65]
            # with ones in column 64 (softmax denominator rides the PV matmul).
            kt_sb = [consts.tile([P, S], BF16, name=f"kt{j}") for j in range(NPAIR)]
            v_sb = [
                consts.tile([P, 4, 8, 65], BF16, name=f"v{pp}") for pp in range(NPIECE)
            ]
            for pp in range(NPIECE):
                nc.vector.memset(v_sb[pp][:, :, :, 64], 1.0)

            qts_all = [[None] * NPAIR for _ in range(NPIECE)]
            ctxts_all = [[None] * NPAIR for _ in range(NPIECE)]

            # ---------- emitters (closures for the weave) ----------
            def q_grp(q, j):
                def fn(q=q, j=j):
                    ps = ps_mm.tile([P, W], F32, tag="mm")
                    for kc in range(NKC):
                        nc.tensor.matmul(
                            ps[:],
                            wq_sb[:, kc, P * j : P * j + P],
                            xt_ap(q, kc),
                            start=(kc == 0),
                            stop=(kc == NKC - 1),
                        )
                    qt = qtp.tile([P, W], BF16, tag=f"qt{j}", name=f"qt{j}_{q}")
                    nc.scalar.copy(qt[:], ps[:])
                    qts_all[q][j] = qt
                return (NKC * W * MM_NS, fn)

            def k_grp(q, j):
                def fn(q=q, j=j):
                    ps = ps_mm.tile([P, W], F32, tag="mm")
                    for kc in range(NKC):
                        nc.tensor.matmul(
                            ps[:],
                            wk_sb[:, kc, P * j : P * j + P],
                            xt_ap(q, kc),
                            start=(kc == 0),
                            stop=(kc == NKC - 1),
                        )
                    nc.vector.tensor_copy(
                        kt_sb[j][:, W * q : W * q + W], ps[:]
                    )
                return (NKC * W * MM_NS, fn)

            def v_grp(q, i):
                def fn(q=q, i=i):
                    ps = ps_mm.tile([P, W], F32, tag="mm")
                    for kc in range(NKC):
                        nc.tensor.matmul(
                            ps[:],
                            xt_ap(q, kc)[:, P * i : P * i + P],
                            wv_sb[:, kc, :],
                            start=(kc == 0),
                            stop=(kc == NKC - 1),
                        )
                    nc.vector.tensor_copy(
                        v_sb[q][:, i, :, 0:64],
                        ps[:].rearrange("q (h d) -> q h d", h=8),
                    )
                return (NKC * W * MM_NS, fn)

            def op_grp(p, si, nsl, st_half=None):
                def fn(p=p, si=si, nsl=nsl, st_half=st_half):
                    if st_half is not None:
                        st_t, half = st_half
                        ps = st_t[:, W * half : W * half + W]
                    else:
                        ps_t = ps_mm.tile([P, W], F32, tag="mm", name=f"op{p}_{si}_{nsl}")
                        ps = ps_t[:]
                    for g in range(NPAIR):
                        nc.tensor.matmul(
                            ps,
                            ctxts_all[p][g][:, P * si : P * si + P],
                            wo_sb[:, g, W * nsl : W * nsl + W],
                            start=(g == 0),
                            stop=(g == NPAIR - 1),
                            skip_group_check=True,
                        )
                    osb = osbp.tile([P, W], F32, tag="osb")
                    if st_half is not None or (p == NPIECE - 1 and (si + nsl) % 2):
                        nc.scalar.copy(osb[:], ps)
                    else:
                        nc.vector.tensor_copy(osb[:], ps)
                    nc.sync.dma_start(
                        out_d[
                            W * p + P * si : W * p + P * si + P,
                            W * nsl : W * nsl + W,
                        ],
                        osb[:],
                    )
                return (NPAIR * W * MM_NS, fn)

            def op_filler(p):
                return [op_grp(p, si, nsl) for si in range(4) for nsl in range(2)]

            # ---------- attention primary stream for piece p ----------
            # k_embed: {pair_j: (after_chunk, (cost, fn))} — K-projection
            # groups inserted right after pair j's last kt read so the kt
            # write's WAR wait resolves immediately.
            def attn_ops(p, k_embed=None, sc_idx=None):
                ops = []
                if sc_idx is None:
                    sc_idx = {}
                nch = 4 * p + 4
                for j in range(NPAIR):
                    ctx_holder = {}
                    pts = {}

                    def mk_sc(p=p, j=j, c=0, pts=pts):
                        def fn(p=p, j=j, c=c, pts=pts):
                            krel = c - 4 * p
                            sk = SKS[krel] if krel >= 0 else 0
                            st = ps_st.tile(
                                [P, 2 * W], F32, tag="st", name=f"st{p}_{j}_{c}"
                            )
                            for hl in range(2):
                                nc.tensor.matmul(
                                    st[:, W * hl + sk : W * hl + W],
                                    kt_sb[j][64 * hl : 64 * hl + 64, P * c : P * c + P],
                                    qts_all[p][j][64 * hl : 64 * hl + 64, sk:W],
                                    start=True,
                                    stop=True,
                                )
                            pt = ptp.tile(
                                [P, 2 * W], BF16, tag="pt", name=f"pt{p}_{j}_{c}"
                            )
                            st3 = st[:].rearrange("q (h n) -> q h n", h=2)
                            pt3 = pt[:].rearrange("q (h n) -> q h n", h=2)
                            nc.scalar.activation(
                                pt3[:, :, sk:W], st3[:, :, sk:W], EXP, scale=0.125
                            )
                            if krel >= 0:
                                nc.vector.tensor_mul(
                                    pt3[:, :, sk : sk + P],
                                    pt3[:, :, sk : sk + P],
                                    maskA[:, None, :].broadcast_to([P, 2, P]),
                                )
                            pts[c] = pt
                        krel = c - 4 * p
                        sk = SKS[krel] if krel >= 0 else 0
                        return (2 * (W - sk) * MM_NS, fn)

                    def mk_pv(p=p, j=j, c=0, pts=pts, ctx_holder=ctx_holder, nch=nch):
                        def fn(p=p, j=j, c=c, pts=pts, ctx_holder=ctx_holder, nch=nch):
                            krel = c - 4 * p
                            sk = SKS[krel] if krel >= 0 else 0
                            if c == 0:
                                ctx_holder["ctx"] = ps_ctx.tile(
                                    [P, 2 * W], F32, tag="ctx", name=f"ctx{p}_{j}"
                                )
                            ctx = ctx_holder["ctx"]
                            pt = pts.pop(c)
                            for hl in range(2):
                                h = 2 * j + hl
                                nc.tensor.matmul(
                                    ctx[0:65, W * hl + sk : W * hl + W],
                                    v_sb[c // 4][:, c % 4, h, :],
                                    pt[:, W * hl + sk : W * hl + W],
                                    start=(c == 0),
                                    stop=(c == nch - 1),
                                    skip_group_check=True,
                                )
                        krel = c - 4 * p
                        sk = SKS[krel] if krel >= 0 else 0
                        return (2 * (W - sk) * MM_NS, fn)

                    def mk_norm(p=p, j=j, ctx_holder=ctx_holder):
                        def fn(p=p, j=j, ctx_holder=ctx_holder):
                            # r row (partition 64) -> SBUF f32r, broadcast to
                            # partitions 0-63 with a K=1 f32r matmul into a
                            # st-ring PSUM tile, reciprocal, then normalize.
                            # hl=1 first so its SBUF-SBUF move starts early.
                            ctx = ctx_holder["ctx"]
                            ctxt = ctxtp.tile(
                                [P, W], F32R, tag=f"ctxt{j}", name=f"ctxt{j}_{p}"
                            )
                            bc = ps_st.tile(
                                [P, 2 * W], F32, tag="st", name=f"bc{p}_{j}"
                            )
                            for hl in (1, 0):
                                rrow = rp.tile(
                                    [P, W], F32R, tag=f"rr{hl}", name=f"rr{hl}_{p}_{j}"
                                )
                                nc.vector.tensor_copy(
                                    rrow[64:65, :], ctx[64:65, W * hl : W * hl + W]
                                )
                                nc.tensor.matmul(
                                    bc[0:64, W * hl : W * hl + W],
                                    ones_r[64:65, :],
                                    rrow[64:65, :],
                                    start=True,
                                    stop=True,
                                    skip_group_check=True,
                                )
                                rbr = rp.tile(
                                    [64, W], F32, tag=f"rb{hl}", name=f"rb{hl}_{p}_{j}"
                                )
                                nc.vector.reciprocal(
                                    rbr[:], bc[0:64, W * hl : W * hl + W]
                                )
                                if hl == 1:
                                    hst = rp.tile(
                                        [64, W], F32R, tag="hst", name=f"hs{p}_{j}"
                                    )
                                    nc.vector.tensor_mul(
                                        hst[:], ctx[0:64, W : 2 * W], rbr[:]
                                    )
                                    nc.gpsimd.dma_start(ctxt[64:128, :], hst[:])
                                else:
                                    nc.vector.tensor_mul(
                                        ctxt[0:64, :], ctx[0:64, 0:W], rbr[:]
                                    )
                            ctxts_all[p][j] = ctxt
                        # pacing cost keeps filler flowing through the
                        # normalize chain's PE-idle window
                        return (2500.0, fn)

                    # scores run one chunk ahead of PV
                    ke = k_embed.get(j) if k_embed else None
                    sc_idx[(j, 0)] = len(ops)
                    ops.append(mk_sc(c=0))
                    if ke and ke[0] == 0:
                        ops.append(ke[1])
                    for c in range(1, nch):
                        sc_idx[(j, c)] = len(ops)
                        ops.append(mk_sc(c=c))
                        if ke and ke[0] == c:
                            ops.append(ke[1])
                        ops.append(mk_pv(c=c - 1))
                    ops.append(mk_pv(c=nch - 1))
                    ops.append(mk_norm())
                # trailing virtual cost: under-pace the filler slightly so a
                # few ready items remain to cover the bundle-boundary
                # normalize chain
                ops.append((3000.0, lambda: None))
                return ops

            def weave(primary, filler):
                # primary: [(cost, fn)]; filler: [(cost, fn, nb, dl)] where
                # nb = earliest primary index (data not ready before), dl =
                # hard emit-before index (program-order correctness).
                tp = sum(c for c, _ in primary) or 1.0
                tf = sum(c for c, _, _, _ in filler)
                done_p = 0.0
                done_f = 0.0
                remaining = list(filler)
                for pi, (c, fn) in enumerate(primary):
                    done_p += c
                    i = 0
                    while i < len(remaining):
                        cf, ff, nb, dl = remaining[i]
                        if dl is not None and dl <= pi:
                            ff()
                            done_f += cf
                            remaining.pop(i)
                        else:
                            i += 1
                    target = tf * ((done_p / tp) ** 1.5)
                    while done_f < target - 1e-9:
                        k = next(
                            (
                                i
                                for i, (cf, ff, nb, dl) in enumerate(remaining)
                                if nb <= pi
                            ),
                            None,
                        )
                        if k is None:
                            break
                        cf, ff, nb, dl = remaining.pop(k)
                        ff()
                        done_f += cf
                    fn()
                for cf, ff, nb, dl in remaining:
                    ff()

            # ---------- program body ----------
            # pre: QKV(0); bundles:
            #   B0: attn(0) + [Q1 V1] + K1 embedded after pair j's last sc
            #   B1: attn(1) + [op0 Q2 V2] + K2 embedded
            #   B2: attn(2) + [op1 Q3]
            #   B3: attn(3) + [V3 op2] + K3 embedded after pair j's sc(11)
            #   tail: op(3), first groups on the idle st ring for depth
            for it in (
                [q_grp(0, j) for j in range(NPAIR)]
                + [v_grp(0, i) for i in range(4)]
                + [k_grp(0, j) for j in range(NPAIR)]
            ):
                it[1]()

            k_embeds = {
                0: {j: (3, k_grp(1, j)) for j in range(NPAIR)},
                1: {j: (7, k_grp(2, j)) for j in range(NPAIR)},
                2: None,
                # K(3)-j after pair j's sc(11): old chunks don't falsely wait
                # on the kt write, the diagonal follows it.
                3: {j: (11, k_grp(3, j)) for j in range(NPAIR)},
            }
            primary = []
            seg = {}
            norm_end = {}
            for p in range(NPIECE):
                sc_idx = {}
                base = len(primary)
                primary += attn_ops(p, k_embeds[p], sc_idx)
                seg[p] = (base, sc_idx)
                norm_end[p] = len(primary)

            filler = []
            for q in range(1, NPIECE):
                base, sci = seg[q]
                for j in range(NPAIR):
                    filler.append(q_grp(q, j) + (0, base + sci[(j, 0)]))
                for i in range(4):
                    filler.append(v_grp(q, i) + (0, base + sci[(0, 4 * q + i)]))
                for it in op_filler(q - 1):
                    filler.append(it + (norm_end[q - 1], None))
            weave(primary, filler)

            # final outproj: one pair of groups rides the st ring (its slot
            # only waits an exp); the rest pipeline on the mm ring — a second
            # st-ring slot would WAR-wait the last bc tile's recips, which
            # sit on the tail critical chain
            groups = [(si, nsl) for si in range(4) for nsl in range(2)]
            for si, nsl in groups:
                op_grp(NPIECE - 1, si, nsl)[1]()

    nc.compile()
    return nc


_program = None
last_results = None


def _get_program():
    global _program
    if _program is None:
        _program = build_program()
    return _program


def kernel(x, Wq, Wk, Wv, Wo, bo):
    global last_results
    x = np.asarray(x, dtype=np.float32)
    Wq = np.asarray(Wq, dtype=np.float32)
    Wk = np.asarray(Wk, dtype=np.float32)
    Wv = np.asarray(Wv, dtype=np.float32)
    Wo = np.asarray(Wo, dtype=np.float32)
    bo = np.asarray(bo, dtype=np.float32)

    import ml_dtypes
    maskA = np.triu(np.ones((P, P), dtype=ml_dtypes.bfloat16))
    ones = np.ones((P, 64), dtype=np.float32)

    nc = _get_program()
    in_maps = []
    for c in range(8):
        b, hg = c // 2, c % 2
        in_maps.append(
            {
                "xT": np.ascontiguousarray(x[b].T).astype(ml_dtypes.bfloat16),
                "wq": np.ascontiguousarray(
                    Wq[:, DH * hg : DH * hg + DH]
                ).astype(ml_dtypes.bfloat16),
                "wk": np.ascontiguousarray(
                    Wk[:, DH * hg : DH * hg + DH]
                ).astype(ml_dtypes.bfloat16),
                "wv": np.ascontiguousarray(
                    Wv[:, DH * hg : DH * hg + DH]
                ).astype(ml_dtypes.bfloat16),
                "wo": np.ascontiguousarray(Wo[DH * hg : DH * hg + DH, :]),
                "maskA": maskA,
                "ones": ones,
            }
        )
    trace = bool(os.environ.get("KERNEL_TRACE"))
    last_results = run_bass_kernel_spmd(
        nc, in_maps, core_ids=list(range(8)), trace=trace
    )
    outs = [r["out"] for r in last_results.results]
    return np.stack([outs[2 * b] + outs[2 * b + 1] + bo for b in range(4)])


# revision 76
# speedup vs baseline: 1.0004x; 1.0004x over previous
"""Multi-head causal attention (B=4, S=2048, D=1024, H=16, HD=64) on 8 TRN2 cores.

Sharding: core c handles (batch b = c//2, head-group hg = c%2 of 8 heads).
Each core computes QKV projections for its 512-dim head slice, transposed-layout
causal attention, and a partial output projection. Host sums the two head-group
partials per batch and adds the bias.

Per-core pipeline (v2 — software-pipelined across pieces):
  - x fed pre-transposed: xT [1024, 2048]; QT/KT/V via bf16 K=128 matmuls.
  - KT per pair stored bf16 [128, S]: head-even dims on partitions 0-63,
    head-odd on 64-127 (no zero padding); scores run as K=64 matmuls with
    base-partition-64 operands for the odd head (tile_position inferred).
  - Exact causal ragged windows SKS=[0,128,256,384]; a single upper-triangular
    maskA multiplies the 128-wide diagonal block of each kept chunk.
  - P^T = exp(ST/8) on ScalarE; ctx^T = V_aug^T @ P^T with V_aug = [V_h | 1]
    (M=65): row 64 accumulates softmax denominators r[q] for free.
  - normalize: r row copied to SBUF (f32r), broadcast to partitions 0-63 by a
    K=1 f32r matmul (213ns) into a st-ring PSUM tile, reciprocal + muls on
    DVE; head-odd ctx rows moved to partitions 64-127 by one SBUF-SBUF DMA
    issued via GpSimd's software DGE.
  - out[s, :] partial = sum_pairs ctxT_pair.T @ Wo_pair (K=128, moving f32r).
  - Engine schedule: QKV matmuls of piece p+1 and the output projection of
    piece p-1 are woven between attention matmuls of piece p in program order
    so TensorE never starves while ScalarE runs the exp chain.
"""

import os
import numpy as np

import concourse.bass as bass
import concourse.mybir as mybir
from concourse import bacc
import concourse.tile as tile
from concourse.bass_utils import run_bass_kernel_spmd

F32 = mybir.dt.float32
F32R = mybir.dt.float32r
BF16 = mybir.dt.bfloat16
EXP = mybir.ActivationFunctionType.Exp

P = 128
S = 2048
DIN = 1024
DH = 512          # per-core d_out slice (8 heads x 64)
NKC = DIN // P    # 8 contraction chunks
NPAIR = 4         # head pairs per core
NPIECE = S // 512 # 4 q pieces
W = 512

# exact causal ragged start offsets for diagonal chunks (krel = chunk - 4*piece)
SKS = [0, 128, 256, 384]

MM_NS = 0.4167  # PE ns/col, used only for weave pacing


def build_program() -> bass.Bass:
    nc = bacc.Bacc("TRN2", target_bir_lowering=False)

    xT_d = nc.dram_tensor("xT", [DIN, S], BF16, kind="ExternalInput")
    wq_d = nc.dram_tensor("wq", [DIN, DH], BF16, kind="ExternalInput")
    wk_d = nc.dram_tensor("wk", [DIN, DH], BF16, kind="ExternalInput")
    wv_d = nc.dram_tensor("wv", [DIN, DH], BF16, kind="ExternalInput")
    wo_d = nc.dram_tensor("wo", [DH, DIN], F32, kind="ExternalInput")
    maskA_d = nc.dram_tensor("maskA", [P, P], BF16, kind="ExternalInput")
    ones_d = nc.dram_tensor("ones", [P, 64], F32, kind="ExternalInput")
    out_d = nc.dram_tensor("out", [S, DIN], F32, kind="ExternalOutput")

    with tile.TileContext(nc) as tc:
        with (
            tc.tile_pool(name="consts", bufs=1) as consts,
            tc.tile_pool(name="xtp", bufs=4) as xtp,
            tc.tile_pool(name="qtp", bufs=4) as qtp,
            tc.tile_pool(name="ptp", bufs=6) as ptp,
            tc.tile_pool(name="ctxtp", bufs=2) as ctxtp,
            tc.tile_pool(name="rp", bufs=4) as rp,
            tc.tile_pool(name="osbp", bufs=8) as osbp,
            tc.tile_pool(name="ps_st", bufs=2, space="PSUM") as ps_st,
            tc.tile_pool(name="ps_ctx", bufs=1, space="PSUM") as ps_ctx,
            tc.tile_pool(name="ps_mm", bufs=2, space="PSUM") as ps_mm,
        ):
            # ---- input/weight DMAs, batched one per tensor, issued on SP ----
            xT_r = xT_d.rearrange("(kc p) s -> p kc s", p=P)
            xts = []
            wq_sb = consts.tile([P, NKC, DH], BF16)
            wk_sb = consts.tile([P, NKC, DH], BF16)
            wv_sb = consts.tile([P, NKC, DH], BF16)
            wo_sb = consts.tile([P, NPAIR, DIN], F32R)
            maskA = consts.tile([P, P], BF16)

            wq_r = wq_d.rearrange("(kc p) d -> p kc d", p=P)
            nc.sync.dma_start(
                wq_sb[:, 0 : NKC // 2, 0:P], wq_r[:, 0 : NKC // 2, 0:P]
            )
            # piece-0 x split so the first Q matmuls start as soon as the
            # first contraction chunks land
            xt0a1 = xtp.tile([P, 2, W], BF16, tag="xa1", name="xt0a1")
            xt0a2 = xtp.tile([P, 2, W], BF16, tag="xa2", name="xt0a2")
            xt0b = xtp.tile([P, NKC // 2, W], BF16, tag="xtb", name="xt0b")
            nc.sync.dma_start(xt0a1[:], xT_r[:, 0:2, 0:W])
            nc.sync.dma_start(xt0a2[:], xT_r[:, 2:4, 0:W])
            nc.sync.dma_start(
                wq_sb[:, NKC // 2 : NKC, 0:P], wq_r[:, NKC // 2 : NKC, 0:P]
            )
            nc.sync.dma_start(xt0b[:], xT_r[:, NKC // 2 : NKC, 0:W])
            xts.append((xt0a1, xt0a2, xt0b))
            for j in range(1, NPAIR):
                nc.sync.dma_start(
                    wq_sb[:, :, P * j : P * j + P], wq_r[:, :, P * j : P * j + P]
                )
            nc.sync.dma_start(wv_sb[:], wv_d.rearrange("(kc p) d -> p kc d", p=P))
            nc.sync.dma_start(wk_sb[:], wk_d.rearrange("(kc p) d -> p kc d", p=P))
            for p in range(1, NPIECE):
                xa1 = xtp.tile([P, 2, W], BF16, tag="xa1", name=f"xt{p}a1")
                xa2 = xtp.tile([P, 2, W], BF16, tag="xa2", name=f"xt{p}a2")
                xb = xtp.tile([P, NKC // 2, W], BF16, tag="xtb", name=f"xt{p}b")
                nc.sync.dma_start(xa1[:], xT_r[:, 0:2, W * p : W * p + W])
                nc.sync.dma_start(xa2[:], xT_r[:, 2:4, W * p : W * p + W])
                nc.sync.dma_start(xb[:], xT_r[:, NKC // 2 : NKC, W * p : W * p + W])
                xts.append((xa1, xa2, xb))
            nc.sync.dma_start(
                wo_sb[:], wo_d.rearrange("(g p) d -> p g d", p=P).bitcast(F32R)
            )
            nc.sync.dma_start(maskA[:], maskA_d[:])
            ones_r = consts.tile([P, 64], F32R)
            nc.sync.dma_start(ones_r[:], ones_d[:].bitcast(F32R))

            def xt_ap(q, kc):
                if kc < 2:
                    return xts[q][0][:, kc, :]
                if kc < 4:
                    return xts[q][1][:, kc - 2, :]
                return xts[q][2][:, kc - 4, :]

            # KT per pair [128, S]: head-even dims on partitions 0-63,
            # head-odd on 64-127.  V per piece [128, s-chunk(4), head(8), 65]
            # with ones in column 64 (softmax denominator rides the PV matmul).
            kt_sb = [consts.tile([P, S], BF16, name=f"kt{j}") for j in range(NPAIR)]
            v_sb = [
                consts.tile([P, 4, 8, 65], BF16, name=f"v{pp}") for pp in range(NPIECE)
            ]
            for pp in range(NPIECE):
                nc.vector.memset(v_sb[pp][:, :, :, 64], 1.0)

            qts_all = [[None] * NPAIR for _ in range(NPIECE)]
            ctxts_all = [[None] * NPAIR for _ in range(NPIECE)]

            # ---------- emitters (closures for the weave) ----------
            def q_grp(q, j):
                def fn(q=q, j=j):
                    ps = ps_mm.tile([P, W], F32, tag="mm")
                    for kc in range(NKC):
                        nc.tensor.matmul(
                            ps[:],
                            wq_sb[:, kc, P * j : P * j + P],
                            xt_ap(q, kc),
                            start=(kc == 0),
                            stop=(kc == NKC - 1),
                        )
                    qt = qtp.tile([P, W], BF16, tag=f"qt{j}", name=f"qt{j}_{q}")
                    nc.scalar.copy(qt[:], ps[:])
                    qts_all[q][j] = qt
                return (NKC * W * MM_NS, fn)

            def k_grp(q, j):
                def fn(q=q, j=j):
                    ps = ps_mm.tile([P, W], F32, tag="mm")
                    for kc in range(NKC):
                        nc.tensor.matmul(
                            ps[:],
                            wk_sb[:, kc, P * j : P * j + P],
                            xt_ap(q, kc),
                            start=(kc == 0),
                            stop=(kc == NKC - 1),
                        )
                    nc.vector.tensor_copy(
                        kt_sb[j][:, W * q : W * q + W], ps[:]
                    )
                return (NKC * W * MM_NS, fn)

            def v_grp(q, i):
                def fn(q=q, i=i):
                    ps = ps_mm.tile([P, W], F32, tag="mm")
                    for kc in range(NKC):
                        nc.tensor.matmul(
                            ps[:],
                            xt_ap(q, kc)[:, P * i : P * i + P],
                            wv_sb[:, kc, :],
                            start=(kc == 0),
                            stop=(kc == NKC - 1),
                        )
                    nc.vector.tensor_copy(
                        v_sb[q][:, i, :, 0:64],
                        ps[:].rearrange("q (h d) -> q h d", h=8),
                    )
                return (NKC * W * MM_NS, fn)

            def op_grp(p, si, nsl, st_half=None):
                def fn(p=p, si=si, nsl=nsl, st_half=st_half):
                    if st_half is not None:
                        st_t, half = st_half
                        ps = st_t[:, W * half : W * half + W]
                    else:
                        ps_t = ps_mm.tile([P, W], F32, tag="mm", name=f"op{p}_{si}_{nsl}")
                        ps = ps_t[:]
                    for g in range(NPAIR):
                        nc.tensor.matmul(
                            ps,
                            ctxts_all[p][g][:, P * si : P * si + P],
                            wo_sb[:, g, W * nsl : W * nsl + W],
                            start=(g == 0),
                            stop=(g == NPAIR - 1),
                            skip_group_check=True,
                        )
                    osb = osbp.tile([P, W], F32, tag="osb")
                    if st_half is not None or (p == NPIECE - 1 and (si + nsl) % 2):
                        nc.scalar.copy(osb[:], ps)
                    else:
                        nc.vector.tensor_copy(osb[:], ps)
                    nc.sync.dma_start(
                        out_d[
                            W * p + P * si : W * p + P * si + P,
                            W * nsl : W * nsl + W,
                        ],
                        osb[:],
                    )
                return (NPAIR * W * MM_NS, fn)

            def op_filler(p):
                return [op_grp(p, si, nsl) for si in range(4) for nsl in range(2)]

            # ---------- attention primary stream for piece p ----------
            # k_embed: {pair_j: (after_chunk, (cost, fn))} — K-projection
            # groups inserted right after pair j's last kt read so the kt
            # write's WAR wait resolves immediately.
            def attn_ops(p, k_embed=None, sc_idx=None):
                ops = []
                if sc_idx is None:
                    sc_idx = {}
                nch = 4 * p + 4
                for j in range(NPAIR):
                    ctx_holder = {}
                    pts = {}

                    def mk_sc(p=p, j=j, c=0, pts=pts):
                        def fn(p=p, j=j, c=c, pts=pts):
                            krel = c - 4 * p
                            sk = SKS[krel] if krel >= 0 else 0
                            st = ps_st.tile(
                                [P, 2 * W], F32, tag="st", name=f"st{p}_{j}_{c}"
                            )
                            for hl in range(2):
                                nc.tensor.matmul(
                                    st[:, W * hl + sk : W * hl + W],
                                    kt_sb[j][64 * hl : 64 * hl + 64, P * c : P * c + P],
                                    qts_all[p][j][64 * hl : 64 * hl + 64, sk:W],
                                    start=True,
                                    stop=True,
                                )
                            pt = ptp.tile(
                                [P, 2 * W], BF16, tag="pt", name=f"pt{p}_{j}_{c}"
                            )
                            st3 = st[:].rearrange("q (h n) -> q h n", h=2)
                            pt3 = pt[:].rearrange("q (h n) -> q h n", h=2)
                            nc.scalar.activation(
                                pt3[:, :, sk:W], st3[:, :, sk:W], EXP, scale=0.125
                            )
                            if krel >= 0:
                                nc.vector.tensor_mul(
                                    pt3[:, :, sk : sk + P],
                                    pt3[:, :, sk : sk + P],
                                    maskA[:, None, :].broadcast_to([P, 2, P]),
                                )
                            pts[c] = pt
                        krel = c - 4 * p
                        sk = SKS[krel] if krel >= 0 else 0
                        return (2 * (W - sk) * MM_NS, fn)

                    def mk_pv(p=p, j=j, c=0, pts=pts, ctx_holder=ctx_holder, nch=nch):
                        def fn(p=p, j=j, c=c, pts=pts, ctx_holder=ctx_holder, nch=nch):
                            krel = c - 4 * p
                            sk = SKS[krel] if krel >= 0 else 0
                            if c == 0:
                                ctx_holder["ctx"] = ps_ctx.tile(
                                    [P, 2 * W], F32, tag="ctx", name=f"ctx{p}_{j}"
                                )
                            ctx = ctx_holder["ctx"]
                            pt = pts.pop(c)
                            for hl in range(2):
                                h = 2 * j + hl
                                nc.tensor.matmul(
                                    ctx[0:65, W * hl + sk : W * hl + W],
                                    v_sb[c // 4][:, c % 4, h, :],
                                    pt[:, W * hl + sk : W * hl + W],
                                    start=(c == 0),
                                    stop=(c == nch - 1),
                                    skip_group_check=True,
                                )
                        krel = c - 4 * p
                        sk = SKS[krel] if krel >= 0 else 0
                        return (2 * (W - sk) * MM_NS, fn)

                    def mk_norm(p=p, j=j, ctx_holder=ctx_holder):
                        def fn(p=p, j=j, ctx_holder=ctx_holder):
                            # r row (partition 64) -> SBUF f32r, broadcast to
                            # partitions 0-63 with a K=1 f32r matmul into a
                            # st-ring PSUM tile, reciprocal, then normalize.
                            # hl=1 first so its SBUF-SBUF move starts early.
                            ctx = ctx_holder["ctx"]
                            ctxt = ctxtp.tile(
                                [P, W], F32R, tag=f"ctxt{j}", name=f"ctxt{j}_{p}"
                            )
                            bc = ps_st.tile(
                                [P, 2 * W], F32, tag="st", name=f"bc{p}_{j}"
                            )
                            for hl in (1, 0):
                                rrow = rp.tile(
                                    [P, W], F32R, tag=f"rr{hl}", name=f"rr{hl}_{p}_{j}"
                                )
                                nc.vector.tensor_copy(
                                    rrow[64:65, :], ctx[64:65, W * hl : W * hl + W]
                                )
                                nc.tensor.matmul(
                                    bc[0:64, W * hl : W * hl + W],
                                    ones_r[64:65, :],
                                    rrow[64:65, :],
                                    start=True,
                                    stop=True,
                                    skip_group_check=True,
                                )
                                rbr = rp.tile(
                                    [64, W], F32, tag=f"rb{hl}", name=f"rb{hl}_{p}_{j}"
                                )
                                nc.vector.reciprocal(
                                    rbr[:], bc[0:64, W * hl : W * hl + W]
                                )
                                if hl == 1:
                                    hst = rp.tile(
                                        [64, W], F32R, tag="hst", name=f"hs{p}_{j}"
                                    )
                                    nc.vector.tensor_mul(
                                        hst[:], ctx[0:64, W : 2 * W], rbr[:]
                                    )
                                    nc.gpsimd.dma_start(ctxt[64:128, :], hst[:])
                                else:
                                    nc.vector.tensor_mul(
                                        ctxt[0:64, :], ctx[0:64, 0:W], rbr[:]
                                    )
                            ctxts_all[p][j] = ctxt
                        # pacing cost keeps filler flowing through the
                        # normalize chain's PE-idle window
                        return (2500.0, fn)

                    # scores run one chunk ahead of PV
                    ke = k_embed.get(j) if k_embed else None
                    sc_idx[(j, 0)] = len(ops)
                    ops.append(mk_sc(c=0))
                    if ke and ke[0] == 0:
                        ops.append(ke[1])
                    for c in range(1, nch):
                        sc_idx[(j, c)] = len(ops)
                        ops.append(mk_sc(c=c))
                        if ke and ke[0] == c:
                            ops.append(ke[1])
                        ops.append(mk_pv(c=c - 1))
                    ops.append(mk_pv(c=nch - 1))
                    ops.append(mk_norm())
                # trailing virtual cost: under-pace the filler slightly so a
                # few ready items remain to cover the bundle-boundary
                # normalize chain
                ops.append((3000.0, lambda: None))
                return ops

            def weave(primary, filler):
                # primary: [(cost, fn)]; filler: [(cost, fn, nb, dl)] where
                # nb = earliest primary index (data not ready before), dl =
                # hard emit-before index (program-order correctness).
                tp = sum(c for c, _ in primary) or 1.0
                tf = sum(c for c, _, _, _ in filler)
                done_p = 0.0
                done_f = 0.0
                remaining = list(filler)
                for pi, (c, fn) in enumerate(primary):
                    done_p += c
                    i = 0
                    while i < len(remaining):
                        cf, ff, nb, dl = remaining[i]
                        if dl is not None and dl <= pi:
                            ff()
                            done_f += cf
                            remaining.pop(i)
                        else:
                            i += 1
                    target = tf * ((done_p / tp) ** 1.5)
                    while done_f < target - 1e-9:
                        k = next(
                            (
                                i
                                for i, (cf, ff, nb, dl) in enumerate(remaining)
                                if nb <= pi
                            ),
                            None,
                        )
                        if k is None:
                            break
                        cf, ff, nb, dl = remaining.pop(k)
                        ff()
                        done_f += cf
                    fn()
                for cf, ff, nb, dl in remaining:
                    ff()

            # ---------- program body ----------
            # pre: QKV(0); bundles:
            #   B0: attn(0) + [Q1 V1] + K1 embedded after pair j's last sc
            #   B1: attn(1) + [op0 Q2 V2] + K2 embedded
            #   B2: attn(2) + [op1 Q3]
            #   B3: attn(3) + [V3 op2] + K3 embedded after pair j's sc(11)
            #   tail: op(3), first groups on the idle st ring for depth
            for it in (
                [q_grp(0, j) for j in range(NPAIR)]
                + [v_grp(0, i) for i in range(4)]
                + [k_grp(0, j) for j in range(NPAIR)]
            ):
                it[1]()

            k_embeds = {
                0: {j: (3, k_grp(1, j)) for j in range(NPAIR)},
                1: {j: (7, k_grp(2, j)) for j in range(NPAIR)},
                2: None,
                # K(3)-j after pair j's sc(11): old chunks don't falsely wait
                # on the kt write, the diagonal follows it.
                3: {j: (11, k_grp(3, j)) for j in range(NPAIR)},
            }
            primary = []
            seg = {}
            norm_end = {}
            for p in range(NPIECE):
                sc_idx = {}
                base = len(primary)
                primary += attn_ops(p, k_embeds[p], sc_idx)
                seg[p] = (base, sc_idx)
                norm_end[p] = len(primary)

            filler = []
            for q in range(1, NPIECE):
                base, sci = seg[q]
                for j in range(NPAIR):
                    filler.append(q_grp(q, j) + (0, base + sci[(j, 0)]))
                for i in range(4):
                    filler.append(v_grp(q, i) + (0, base + sci[(0, 4 * q + i)]))
                for it in op_filler(q - 1):
                    filler.append(it + (norm_end[q - 1], None))
            weave(primary, filler)

            # final outproj: one pair of groups rides the st ring (its slot
            # only waits an exp); the rest pipeline on the mm ring — a second
            # st-ring slot would WAR-wait the last bc tile's recips, which
            # sit on the tail critical chain
            groups = [(si, nsl) for si in range(4) for nsl in range(2)]
            for si, nsl in groups:
                op_grp(NPIECE - 1, si, nsl)[1]()

    nc.compile()
    return nc


_program = None
last_results = None


def _get_program():
    global _program
    if _program is None:
        _program = build_program()
    return _program


def kernel(x, Wq, Wk, Wv, Wo, bo):
    global last_results
    x = np.asarray(x, dtype=np.float32)
    Wq = np.asarray(Wq, dtype=np.float32)
    Wk = np.asarray(Wk, dtype=np.float32)
    Wv = np.asarray(Wv, dtype=np.float32)
    Wo = np.asarray(Wo, dtype=np.float32)
    bo = np.asarray(bo, dtype=np.float32)

    import ml_dtypes
    maskA = np.triu(np.ones((P, P), dtype=ml_dtypes.bfloat16))
    ones = np.ones((P, 64), dtype=np.float32)

    nc = _get_program()
    in_maps = []
    for c in range(8):
        b, hg = c // 2, c % 2
        in_maps.append(
            {
                "xT": np.ascontiguousarray(x[b].T).astype(ml_dtypes.bfloat16),
                "wq": np.ascontiguousarray(
                    Wq[:, DH * hg : DH * hg + DH]
                ).astype(ml_dtypes.bfloat16),
                "wk": np.ascontiguousarray(
                    Wk[:, DH * hg : DH * hg + DH]
                ).astype(ml_dtypes.bfloat16),
                "wv": np.ascontiguousarray(
                    Wv[:, DH * hg : DH * hg + DH]
                ).astype(ml_dtypes.bfloat16),
                "wo": np.ascontiguousarray(Wo[DH * hg : DH * hg + DH, :]),
                "maskA": maskA,
                "ones": ones,
            }
        )
    trace = bool(os.environ.get("KERNEL_TRACE"))
    last_results = run_bass_kernel_spmd(
        nc, in_maps, core_ids=list(range(8)), trace=trace
    )
    outs = [r["out"] for r in last_results.results]
    return np.stack([outs[2 * b] + outs[2 * b + 1] + bo for b in range(4)])
